# revision 11
# baseline (speedup 1.0000x reference)
"""Trainium2 Bass kernel for nn_Attn_25409026523783.

Dense causal multi-head attention block (B=64, S=256, D=2048, H=16, HD=128):
    qkv = x @ w_qkv.T ; causal softmax attention per head ; out = ctx @ w_o.T

Strategy:
  - Batch-shard across the 8 NeuronCores (8 batches per core). No collectives:
    the host scatters inputs and concatenates per-core outputs.
  - Host pre-transposes x, w_qkv, w_o so every matmul operand DMA-loads with
    the contraction dim on partitions (fp32 transposing DMA is ~19x slow).
  - All matmuls run as float32r (FP22 multiply, fp32 accumulate): 1 cycle/row
    at free-dim >= 256, i.e. bf16 speed with ~6e-5 relative precision.
  - Per core, three phases over one TileContext:
      1) QKV projection (weights streamed once, x^T resident in SBUF), Q/K
         produced feature-major ([e, t]) and V token-major ([t, f]); spilled
         to DRAM scratch.
      2) Attention per (batch, head) in transposed layout S^T[k, q]: exp via
         ACT (scale folded), multiplicative causal mask, denominator via
         ones-matmul, reciprocal broadcast via 1-partition matmul, ctx^T
         accumulated directly from V (token-major) x A^T.
      3) Output projection with w_o^T resident, ctx^T tiles as stationary;
         result [t, e] streamed out.
"""

import os
import sys

import numpy as np

for _p in ("/opt/trn_rl_repo", "/root/.axon_site/_ro/trn_rl_repo"):
    if os.path.isdir(_p) and _p not in sys.path:
        sys.path.insert(0, _p)

import concourse.bass as bass  # noqa: E402
import concourse.mybir as mybir  # noqa: E402
import concourse.tile as tile  # noqa: E402
from concourse import bacc  # noqa: E402
from concourse.bass_utils import run_bass_kernel_spmd  # noqa: E402

F32 = mybir.dt.float32
F32R = mybir.dt.float32r

# Problem constants (per spec, hardcoded)
B, S, D, H = 64, 256, 2048, 16
HD = D // H  # 128
N_CORES = 8
NB = B // N_CORES           # 8 batches per core
T = NB * S                  # 2048 tokens per core
EQK = 2 * D                 # 4096 Q+K feature rows
P = 128
SCALE = float(HD) ** -0.5

_CACHE = {}


def _build():
    """Build + compile the per-core Bass program (same program on all cores)."""
    nc = bacc.Bacc("TRN2", target_bir_lowering=False, debug=False,
                   enable_asserts=False)

    xT = nc.dram_tensor("xT", [D, T], F32, kind="ExternalInput").ap()
    wqkvT = nc.dram_tensor("wqkvT", [D, 3 * D], F32, kind="ExternalInput").ap()
    woT = nc.dram_tensor("woT", [D, D], F32, kind="ExternalInput").ap()
    mask = nc.dram_tensor("mask", [P, P], F32, kind="ExternalInput").ap()
    out = nc.dram_tensor("out", [T, D], F32, kind="ExternalOutput").ap()

    qkT_d = nc.dram_tensor("qkT_d", [EQK, T], F32).ap()   # Q^T/K^T [e, t]
    v_d = nc.dram_tensor("v_d", [T, D], F32).ap()         # V [t, f]

    with tile.TileContext(nc) as tc:
        # ---------------- Phase 1: QKV projection ----------------
        with tc.tile_pool(name="xt", bufs=1) as xt_pool, \
             tc.tile_pool(name="p1out", bufs=6) as o_pool, \
             tc.tile_pool(name="p1psum", bufs=4, space="PSUM") as ps_pool:

            # Resident x^T: [128, 16 d-tiles, 2048 t] (128 KiB/partition)
            xt = xt_pool.tile([P, D // P, T], F32R)
            nc.sync.dma_start(out=xt[:], in_=xT.rearrange("(a p) t -> p a t", p=P).bitcast(F32R))

            copy_i = 0
            # --- 1a: Q/K features, output [e, t] ---
            with tc.tile_pool(name="wqk", bufs=2) as w_pool:
                for eg in range(EQK // 256):     # 16 e-groups of 256
                    wg = w_pool.tile([P, D // P, 256], F32R, tag="wqk")
                    nc.sync.dma_start(
                        out=wg[:],
                        in_=wqkvT[:, eg * 256:(eg + 1) * 256]
                            .rearrange("(a p) e -> p a e", p=P).bitcast(F32R),
                    )
                    for es in range(2):          # two 128-row e-tiles per group
                        e0 = eg * 256 + es * P
                        for tch in range(T // 512):   # 4 token chunks of 512
                            ps = ps_pool.tile([P, 512], F32)
                            for dt_ in range(D // P):
                                nc.tensor.matmul(
                                    ps[:],
                                    wg[:, dt_, es * P:(es + 1) * P],
                                    xt[:, dt_, tch * 512:(tch + 1) * 512],
                                    start=(dt_ == 0), stop=(dt_ == D // P - 1),
                                )
                            ot = o_pool.tile([P, 512], F32)
                            if copy_i % 2 == 0:
                                nc.vector.tensor_copy(ot[:], ps[:])
                            else:
                                nc.scalar.copy(ot[:], ps[:])
                            copy_i += 1
                            nc.sync.dma_start(
                                out=qkT_d[e0:e0 + P, tch * 512:(tch + 1) * 512],
                                in_=ot[:],
                            )

            # --- 1b: V features, output [t, f] ---
            with tc.tile_pool(name="wv", bufs=2) as w_pool2:
                for fg in range(D // 512):       # 4 f-groups of 512
                    wg2 = w_pool2.tile([P, D // P, 512], F32R, tag="wv")
                    nc.sync.dma_start(
                        out=wg2[:],
                        in_=wqkvT[:, EQK + fg * 512:EQK + (fg + 1) * 512]
                            .rearrange("(a p) e -> p a e", p=P).bitcast(F32R),
                    )
                    for tt in range(T // P):     # 16 token tiles of 128
                        ps = ps_pool.tile([P, 512], F32)
                        for dt_ in range(D // P):
                            nc.tensor.matmul(
                                ps[:],
                                xt[:, dt_, tt * P:(tt + 1) * P],
                                wg2[:, dt_, :],
                                start=(dt_ == 0), stop=(dt_ == D // P - 1),
                            )
                        ot = o_pool.tile([P, 512], F32)
                        if copy_i % 2 == 0:
                            nc.vector.tensor_copy(ot[:], ps[:])
                        else:
                            nc.scalar.copy(ot[:], ps[:])
                        copy_i += 1
                        nc.sync.dma_start(
                            out=v_d[tt * P:(tt + 1) * P, fg * 512:(fg + 1) * 512],
                            in_=ot[:],
                        )

        # ---------------- Phase 2+3: attention + out-projection ----------------
        HH = 8  # heads per load group (half batch)
        with tc.tile_pool(name="const", bufs=1) as c_pool, \
             tc.tile_pool(name="wo", bufs=1) as wo_pool, \
             tc.tile_pool(name="qkv", bufs=2) as qkv_pool, \
             tc.tile_pool(name="at", bufs=2) as a_pool, \
             tc.tile_pool(name="ctx", bufs=1) as ctx_pool, \
             tc.tile_pool(name="aux", bufs=2) as aux_pool, \
             tc.tile_pool(name="p3out", bufs=2) as o3_pool, \
             tc.tile_pool(name="ps_s", bufs=2, space="PSUM") as pss_pool, \
             tc.tile_pool(name="ps_aux", bufs=2, space="PSUM") as psa_pool, \
             tc.tile_pool(name="ps_c", bufs=2, space="PSUM") as psc_pool, \
             tc.tile_pool(name="ps_o", bufs=2, space="PSUM") as pso_pool:

            maskt = c_pool.tile([P, P], F32R)
            nc.sync.dma_start(out=maskt[:], in_=mask[:, :].bitcast(F32R))
            # mask col 127 is all-ones [128,1]; mask row 0 is all-ones [1,128]
            ones_col = maskt[:, P - 1:P]
            ones_row = maskt[0:1, :]

            # Resident w_o^T: [128, 16 d'-tiles, 2048 e] (128 KiB/partition)
            wo = wo_pool.tile([P, D // P, D], F32R)
            nc.sync.dma_start(out=wo[:], in_=woT.rearrange("(a p) e -> p a e", p=P).bitcast(F32R))

            # ctx^T for one batch: [128 d', 16 h, 256 q]
            ctx_b = ctx_pool.tile([P, H, S], F32R)

            for b in range(NB):
                t0 = b * S
                for half in range(H // HH):
                    h0 = half * HH
                    q_t = qkv_pool.tile([P, HH, S], F32R, tag="q")
                    nc.sync.dma_start(
                        out=q_t[:],
                        in_=qkT_d[h0 * HD:(h0 + HH) * HD, t0:t0 + S]
                            .rearrange("(h p) t -> p h t", p=P).bitcast(F32R),
                    )
                    k_t = qkv_pool.tile([P, HH, S], F32R, tag="k")
                    nc.sync.dma_start(
                        out=k_t[:],
                        in_=qkT_d[D + h0 * HD:D + (h0 + HH) * HD, t0:t0 + S]
                            .rearrange("(h p) t -> p h t", p=P).bitcast(F32R),
                    )
                    v_t = qkv_pool.tile([P, 2, HH, HD], F32R, tag="v")
                    nc.sync.dma_start(
                        out=v_t[:],
                        in_=v_d[t0:t0 + S, h0 * HD:(h0 + HH) * HD]
                            .rearrange("(kt p) (h f) -> p kt h f", p=P, h=HH).bitcast(F32R),
                    )
                    for hh in range(HH):
                        h = h0 + hh
                        # S^T[k, q] for both k-tiles, one PSUM bank
                        ps_s = pss_pool.tile([P, 2, S], F32)
                        nc.tensor.matmul(
                            ps_s[:, 0, :], k_t[:, hh, 0:P], q_t[:, hh, :],
                            start=True, stop=True)
                        nc.tensor.matmul(
                            ps_s[:, 1, :], k_t[:, hh, P:S], q_t[:, hh, :],
                            start=True, stop=True)
                        # A^T = exp(scale * S^T)
                        a_t = a_pool.tile([P, 2, S], F32R)
                        nc.scalar.activation(
                            a_t[:, 0, :], ps_s[:, 0, :],
                            mybir.ActivationFunctionType.Exp, scale=SCALE)
                        nc.scalar.activation(
                            a_t[:, 1, :], ps_s[:, 1, :],
                            mybir.ActivationFunctionType.Exp, scale=SCALE)
                        # causal mask (multiplicative)
                        nc.vector.tensor_mul(
                            a_t[:, 0, 0:P], a_t[:, 0, 0:P], maskt[:])
                        nc.vector.tensor_scalar_mul(a_t[:, 1, 0:P], a_t[:, 1, 0:P], 0.0)
                        nc.vector.tensor_mul(
                            a_t[:, 1, P:S], a_t[:, 1, P:S], maskt[:])
                        # denominators: [1, 256] = ones^T @ A^T, both k-tiles
                        ps_aux = psa_pool.tile([P, 512], F32)
                        nc.tensor.matmul(
                            ps_aux[0:1, S:2 * S], ones_col, a_t[:, 0, :],
                            start=True, stop=False)
                        nc.tensor.matmul(
                            ps_aux[0:1, S:2 * S], ones_col, a_t[:, 1, :],
                            start=False, stop=True)
                        rcp = aux_pool.tile([1, S], F32R, tag="rcp")
                        with nc.allow_low_precision(
                                reason="f32r softmax normalizer (fp22 ok)"):
                            nc.vector.reciprocal(rcp[:], ps_aux[0:1, S:2 * S])
                        # broadcast 1/denom across 128 partitions via PE
                        nc.tensor.matmul(
                            ps_aux[:, 0:S], ones_row, rcp[:],
                            start=True, stop=True)
                        rb = aux_pool.tile([P, S], F32, tag="rb")
                        nc.vector.tensor_copy(rb[:], ps_aux[:, 0:S])
                        # ctx^T[d', q] = V^T-free accumulation over k-tiles
                        ps_c = psc_pool.tile([P, S], F32)
                        nc.tensor.matmul(
                            ps_c[:], v_t[:, 0, hh, :], a_t[:, 0, :],
                            start=True, stop=False)
                        nc.tensor.matmul(
                            ps_c[:], v_t[:, 1, hh, :], a_t[:, 1, :],
                            start=False, stop=True)
                        nc.vector.tensor_mul(ctx_b[:, h, :], ps_c[:], rb[:])

                # out-projection for batch b: out[t, e] = ctx^T.T @ wo^T
                for tt in range(S // P):
                    for ec in range(D // 512):
                        ps_o = pso_pool.tile([P, 512], F32)
                        for h in range(H):
                            nc.tensor.matmul(
                                ps_o[:],
                                ctx_b[:, h, tt * P:(tt + 1) * P],
                                wo[:, h, ec * 512:(ec + 1) * 512],
                                start=(h == 0), stop=(h == H - 1),
                            )
                        o_t = o3_pool.tile([P, 512], F32)
                        nc.vector.tensor_copy(o_t[:], ps_o[:])
                        nc.sync.dma_start(
                            out=out[t0 + tt * P:t0 + (tt + 1) * P,
                                    ec * 512:(ec + 1) * 512],
                            in_=o_t[:],
                        )

    nc.compile()
    return nc


def get_nc():
    if "nc" not in _CACHE:
        _CACHE["nc"] = _build()
    return _CACHE["nc"]


def make_in_maps(x, w_qkv, w_o):
    x = np.ascontiguousarray(np.asarray(x, dtype=np.float32))
    w_qkv = np.asarray(w_qkv, dtype=np.float32)
    w_o = np.asarray(w_o, dtype=np.float32)
    wqkvT = np.ascontiguousarray(w_qkv.T)          # [D, 3D]
    woT = np.ascontiguousarray(w_o.T)              # [D, D]
    # mask[k, q] = 1 where k <= q (causal, diagonal kept)
    mask = np.triu(np.ones((P, P), dtype=np.float32))
    in_maps = []
    for c in range(N_CORES):
        xs = x[c * NB:(c + 1) * NB].reshape(T, D)  # [t, d]
        xT_c = np.ascontiguousarray(xs.T)          # [d, t]
        in_maps.append({"xT": xT_c, "wqkvT": wqkvT, "woT": woT, "mask": mask})
    return in_maps


def run(x, w_qkv, w_o, trace=False):
    nc = get_nc()
    in_maps = make_in_maps(x, w_qkv, w_o)
    res = run_bass_kernel_spmd(nc, in_maps, list(range(N_CORES)), trace=trace)
    outs = [res.results[i]["out"].reshape(NB, S, D) for i in range(N_CORES)]
    return np.concatenate(outs, axis=0), res


def kernel(**inputs):
    out, _ = run(inputs["x"], inputs["w_qkv"], inputs["w_o"])
    return out


def bench(x, w_qkv, w_o, iters=12):
    """Per-execution wall time of the sharded NEFF call, minus RPC floor.

    Mirrors bass2jax.run_bass_via_pjrt exactly (incl. donation); donated zero
    output buffers are re-staged outside the timed region each iteration.
    Returns (per_iter_ns, floor_ns, outputs_full).
    """
    import time

    import jax
    from jax.sharding import Mesh, PartitionSpec, NamedSharding
    from jax.experimental.shard_map import shard_map

    import concourse.mybir as mb
    from concourse import bass2jax

    nc = get_nc()
    bass2jax.install_neuronx_cc_hook()
    in_maps = make_in_maps(x, w_qkv, w_o)

    partition_name = (nc.partition_id_tensor.name
                      if nc.partition_id_tensor else None)
    in_names, out_names, out_avals, zero_outs = [], [], [], []
    for alloc in nc.m.functions[0].allocations:
        if not isinstance(alloc, mb.MemoryLocationSet):
            continue
        name = alloc.memorylocations[0].name
        if alloc.kind == "ExternalInput":
            if name != partition_name:
                in_names.append(name)
        elif alloc.kind == "ExternalOutput":
            out_names.append(name)
            shape = tuple(alloc.tensor_shape)
            dtype = mb.dt.np(alloc.dtype)
            out_avals.append(jax.core.ShapedArray(shape, dtype))
            zero_outs.append(np.zeros(shape, dtype))
    n_params = len(in_names)
    n_outs = len(out_names)
    all_names = in_names + out_names
    if partition_name is not None:
        all_names = all_names + [partition_name]
    donate = tuple(range(n_params, n_params + n_outs))

    def _body(*args):
        operands = list(args)
        if partition_name is not None:
            operands.append(bass2jax.partition_id_tensor())
        outs = bass2jax._bass_exec_p.bind(
            *operands,
            out_avals=tuple(out_avals),
            in_names=tuple(all_names),
            out_names=tuple(out_names),
            lowering_input_output_aliases=(),
            sim_require_finite=True,
            sim_require_nnan=True,
            nc=nc,
        )
        return tuple(outs)

    devices = jax.devices()[:N_CORES]
    mesh = Mesh(np.asarray(devices), ("core",))
    n_all = n_params + n_outs
    sharded = jax.jit(
        shard_map(_body, mesh=mesh,
                  in_specs=(PartitionSpec("core"),) * n_all,
                  out_specs=(PartitionSpec("core"),) * n_outs,
                  check_rep=False),
        donate_argnums=donate,
        keep_unused=True,
    )
    sharding = NamedSharding(mesh, PartitionSpec("core"))
    concat_in = [
        np.concatenate([in_maps[c][nm] for c in range(N_CORES)], axis=0)
        for nm in in_names
    ]
    concat_zeros = [np.zeros((N_CORES * z.shape[0], *z.shape[1:]), z.dtype)
                    for z in zero_outs]
    dev_in = [jax.device_put(a, sharding) for a in concat_in]

    # RPC floor: trivial sharded op on the same mesh
    tiny = jax.device_put(np.zeros((N_CORES, 8), np.float32), sharding)
    tiny_fn = jax.jit(shard_map(lambda a: a + 1.0, mesh=mesh,
                                in_specs=(PartitionSpec("core"),),
                                out_specs=PartitionSpec("core"),
                                check_rep=False))
    jax.block_until_ready(tiny_fn(tiny))
    floors = []
    for _ in range(10):
        t0 = time.perf_counter()
        jax.block_until_ready(tiny_fn(tiny))
        floors.append(time.perf_counter() - t0)
    floor_ns = min(floors) * 1e9

    def chain(n):
        """Wall time for n chained executions (outputs donated as next call's
        buffers) under a single blocking wait."""
        dev_zeros = [jax.device_put(a, sharding) for a in concat_zeros]
        jax.block_until_ready(dev_zeros)
        t0 = time.perf_counter()
        outs = sharded(*dev_in, *dev_zeros)
        for _ in range(n - 1):
            outs = sharded(*dev_in, *outs)
        jax.block_until_ready(outs)
        return time.perf_counter() - t0, outs

    chain(1)  # warm-up / compile
    lo_n, hi_n = 2, 2 + iters
    lo = min(chain(lo_n)[0] for _ in range(3))
    hi_t, outs = chain(hi_n)
    for _ in range(2):
        t2, o2 = chain(hi_n)
        if t2 < hi_t:
            hi_t, outs = t2, o2
    per_iter_ns = (hi_t - lo) / (hi_n - lo_n) * 1e9
    print(f"bench: chain({lo_n}) {lo*1e3:.2f} ms, chain({hi_n}) {hi_t*1e3:.2f} ms, "
          f"floor {floor_ns/1e6:.2f} ms")
    out_np = np.asarray(outs[out_names.index("out")])
    per_core = out_np.reshape(N_CORES, T, D)
    full = np.concatenate([per_core[c].reshape(NB, S, D)
                           for c in range(N_CORES)], axis=0)
    return per_iter_ns, floor_ns, full


# revision 14
# speedup vs baseline: 8.5086x; 8.5086x over previous
"""Trainium2 Bass kernel for nn_Attn_25409026523783.

Dense causal multi-head attention block (B=64, S=256, D=2048, H=16, HD=128):
    qkv = x @ w_qkv.T ; causal softmax attention per head ; out = ctx @ w_o.T

Strategy:
  - Batch-shard across the 8 NeuronCores (8 batches per core). No collectives:
    the host scatters inputs and concatenates per-core outputs.
  - Host pre-tiles x, w_qkv, w_o into the exact SBUF stripe layouts so every
    DMA is a large fully-contiguous block transfer (contraction dim lands on
    partitions without any transposing DMA, which is ~19x slow for fp32).
  - All matmuls run as float32r (FP22 multiply, fp32 accumulate): 1 cycle/row
    at free-dim >= 256, i.e. bf16 speed with ~6e-5 relative precision
    (measured ~266 ns per 128x128x512 matmul incl. weight-reload overhead).
  - Per core, three phases over one TileContext:
      1) QKV projection (weights streamed once, x^T resident in SBUF), Q/K
         produced feature-major ([e, t]) and V token-major ([t, f]); spilled
         to DRAM scratch in block layouts matched to phase-2 reads.
      2) Attention per (batch, head) in transposed layout S^T[k, q]: one exp
         over both k-tiles via ACT (scale folded), one multiplicative causal
         mask multiply, denominator via ones-matmul, reciprocal broadcast via
         1-partition matmul, ctx^T accumulated directly from V x A^T.
      3) Output projection with w_o^T resident, ctx^T tiles as stationary;
         result [t, e] streamed out.
"""

import os
import sys

import numpy as np

for _p in ("/opt/trn_rl_repo", "/root/.axon_site/_ro/trn_rl_repo"):
    if os.path.isdir(_p) and _p not in sys.path:
        sys.path.insert(0, _p)

import concourse.bass as bass  # noqa: E402,F401
import concourse.mybir as mybir  # noqa: E402
import concourse.tile as tile  # noqa: E402
from concourse import bacc  # noqa: E402
from concourse.bass_utils import run_bass_kernel_spmd  # noqa: E402

F32 = mybir.dt.float32
F32R = mybir.dt.float32r
EXP = mybir.ActivationFunctionType.Exp

# Problem constants (per spec, hardcoded)
B, S, D, H = 64, 256, 2048, 16
HD = D // H  # 128
N_CORES = 8
NB = B // N_CORES           # 8 batches per core
T = NB * S                  # 2048 tokens per core
P = 128
DT = D // P                 # 16 d-tiles
ET = 2 * D // P             # 32 Q+K feature tiles (Q: 0..15, K: 16..31)
SCALE = float(HD) ** -0.5
HQ = 4                      # heads per attention load group (quarter batch)

_CACHE = {}


def _build(variant="full"):
    """Build + compile the per-core Bass program (same program on all cores).

    variant: "full" | "p1" (projection only) | "p23" (attention+out-proj only,
    with scratch as external inputs) — p1/p23 exist for phase-attribution
    benchmarking only.
    """
    nc = bacc.Bacc("TRN2", target_bir_lowering=False, debug=False,
                   enable_asserts=False)

    # Host-pretiled inputs (see make_in_maps for the exact numpy packing).
    xt_t = nc.dram_tensor("xt_t", [P, DT, T], F32, kind="ExternalInput").ap()
    wqk_t = nc.dram_tensor("wqk_t", [16, P, DT, 256], F32,
                           kind="ExternalInput").ap()
    wv_t = nc.dram_tensor("wv_t", [4, P, DT, 512], F32,
                          kind="ExternalInput").ap()
    wo_t = nc.dram_tensor("wo_t", [P, DT, D], F32, kind="ExternalInput").ap()
    # mask2 [128, 512]: [tri(k<=q) | ones | zeros | tri]
    mask2 = nc.dram_tensor("mask2", [P, 4 * P], F32, kind="ExternalInput").ap()
    out = nc.dram_tensor("out", [T, D], F32, kind="ExternalOutput").ap()

    scratch_kind = "ExternalInput" if variant == "p23" else "Internal"
    # Q^T/K^T blocks: [batch, e-tile, partition(d'), t(256)]
    qkT_d = nc.dram_tensor("qkT_d", [NB, ET, P, S], F32,
                           kind=scratch_kind).ap()
    # V blocks: [t-tile, f-group, partition(k), f(512)]
    v_d = nc.dram_tensor("v_d", [T // P, 4, P, 512], F32,
                         kind=scratch_kind).ap()

    with tile.TileContext(nc) as tc:
        # ---------------- Phase 1: QKV projection ----------------
        if variant != "p23":
         with tc.tile_pool(name="xt", bufs=1) as xt_pool, \
              tc.tile_pool(name="p1out", bufs=6) as o_pool, \
              tc.tile_pool(name="p1psum", bufs=4, space="PSUM") as ps_pool:

            # Resident x^T: [128, 16 d-tiles, 2048 t] (128 KiB/partition),
            # loaded as 16 x 1 MiB contiguous DMAs so compute starts early.
            xt = xt_pool.tile([P, DT, T], F32R)
            for dt_ in range(DT):
                nc.sync.dma_start(out=xt[:, dt_, :],
                                  in_=xt_t[:, dt_, :].bitcast(F32R))

            copy_i = 0
            # --- 1a: Q/K features, output blocks [b, et, p, 256] ---
            with tc.tile_pool(name="wqk", bufs=2) as w_pool:
                for eg in range(16):         # 16 e-groups of 256
                    wg = w_pool.tile([P, DT, 256], F32R, tag="wqk")
                    nc.sync.dma_start(out=wg[:], in_=wqk_t[eg].bitcast(F32R))
                    for es in range(2):      # two 128-row e-tiles per group
                        et = eg * 2 + es
                        for tch in range(T // 512):   # 4 token chunks of 512
                            ps = ps_pool.tile([P, 512], F32)
                            for dt_ in range(DT):
                                nc.tensor.matmul(
                                    ps[:],
                                    wg[:, dt_, es * P:(es + 1) * P],
                                    xt[:, dt_, tch * 512:(tch + 1) * 512],
                                    start=(dt_ == 0), stop=(dt_ == DT - 1),
                                )
                            ot = o_pool.tile([P, 512], F32)
                            if copy_i % 2 == 0:
                                nc.vector.tensor_copy(ot[:], ps[:])
                            else:
                                nc.scalar.copy(ot[:], ps[:])
                            copy_i += 1
                            for half in range(2):
                                nc.sync.dma_start(
                                    out=qkT_d[2 * tch + half, et],
                                    in_=ot[:, half * S:(half + 1) * S],
                                )

            # --- 1b: V features, output blocks [tt, fg, p, 512] ---
            with tc.tile_pool(name="wv", bufs=2) as w_pool2:
                for fg in range(4):          # 4 f-groups of 512
                    wg2 = w_pool2.tile([P, DT, 512], F32R, tag="wv")
                    nc.sync.dma_start(out=wg2[:], in_=wv_t[fg].bitcast(F32R))
                    for tt in range(T // P):     # 16 token tiles of 128
                        ps = ps_pool.tile([P, 512], F32)
                        for dt_ in range(DT):
                            nc.tensor.matmul(
                                ps[:],
                                xt[:, dt_, tt * P:(tt + 1) * P],
                                wg2[:, dt_, :],
                                start=(dt_ == 0), stop=(dt_ == DT - 1),
                            )
                        ot = o_pool.tile([P, 512], F32)
                        if copy_i % 2 == 0:
                            nc.vector.tensor_copy(ot[:], ps[:])
                        else:
                            nc.scalar.copy(ot[:], ps[:])
                        copy_i += 1
                        nc.sync.dma_start(out=v_d[tt, fg], in_=ot[:])

        # -------------- Phase 2+3: attention + out-projection --------------
        if variant == "p1":
            with tc.tile_pool(name="tail", bufs=1) as tail_pool:
                tp = tail_pool.tile([P, 512], F32)
                nc.sync.dma_start(out=tp[:], in_=v_d[0, 0])
                nc.sync.dma_start(out=out[0:P, 0:512], in_=tp[:])
        else:
         with tc.tile_pool(name="const", bufs=1) as c_pool, \
              tc.tile_pool(name="wo", bufs=1) as wo_pool, \
              tc.tile_pool(name="qkv", bufs=2) as qkv_pool, \
              tc.tile_pool(name="at", bufs=3) as a_pool, \
              tc.tile_pool(name="ctx", bufs=2) as ctx_pool, \
              tc.tile_pool(name="aux", bufs=2) as aux_pool, \
              tc.tile_pool(name="p3out", bufs=2) as o3_pool, \
              tc.tile_pool(name="ps_s", bufs=2, space="PSUM") as pss_pool, \
              tc.tile_pool(name="ps_aux", bufs=2, space="PSUM") as psa_pool, \
              tc.tile_pool(name="ps_c", bufs=2, space="PSUM") as psc_pool, \
              tc.tile_pool(name="ps_o", bufs=2, space="PSUM") as pso_pool:

            m2 = c_pool.tile([P, 4 * P], F32R)
            nc.sync.dma_start(out=m2[:], in_=mask2[:, :].bitcast(F32R))
            ones_col = m2[:, 2 * P - 1:2 * P]   # all-ones [128, 1]
            ones_row = m2[0:1, P:2 * P]         # all-ones [1, 128]

            # Resident w_o^T: [128, 16 d'-tiles, 2048 e] (128 KiB/partition)
            wo = wo_pool.tile([P, DT, D], F32R)
            nc.sync.dma_start(out=wo[:], in_=wo_t.bitcast(F32R))

            for b in range(NB):
                t0 = b * S
                # ctx^T for this batch: [128 d', 16 h, 256 q]
                ctx_b = ctx_pool.tile([P, H, S], F32R, tag="ctx", name="ctx_b")
                for qi in range(H // HQ):
                    h0 = qi * HQ
                    q_t = qkv_pool.tile([P, HQ, S], F32R, tag="q")
                    nc.sync.dma_start(
                        out=q_t[:],
                        in_=qkT_d[b, h0:h0 + HQ]
                            .rearrange("e p t -> p e t").bitcast(F32R))
                    k_t = qkv_pool.tile([P, HQ, S], F32R, tag="k")
                    nc.sync.dma_start(
                        out=k_t[:],
                        in_=qkT_d[b, 16 + h0:16 + h0 + HQ]
                            .rearrange("e p t -> p e t").bitcast(F32R))
                    v_t = qkv_pool.tile([P, 2, 512], F32R, tag="v")
                    for kt in range(2):
                        nc.sync.dma_start(
                            out=v_t[:, kt, :],
                            in_=v_d[2 * b + kt, qi].bitcast(F32R))
                    for hh in range(HQ):
                        h = h0 + hh
                        # S^T[k, q] for both k-tiles, one PSUM bank
                        ps_s = pss_pool.tile([P, 2, S], F32)
                        nc.tensor.matmul(
                            ps_s[:, 0, :], k_t[:, hh, 0:P], q_t[:, hh, :],
                            start=True, stop=True)
                        nc.tensor.matmul(
                            ps_s[:, 1, :], k_t[:, hh, P:S], q_t[:, hh, :],
                            start=True, stop=True)
                        # A^T = exp(scale*S^T) * causal mask — one exp, one mul
                        a_t = a_pool.tile([P, 2, S], F32R)
                        nc.scalar.activation(a_t[:, :, :], ps_s[:, :, :],
                                             EXP, scale=SCALE)
                        nc.vector.tensor_mul(
                            a_t[:, :, :], a_t[:, :, :],
                            m2[:, :].rearrange("p (k s) -> p k s", k=2))
                        # denominators [1, 256] = ones^T @ A^T over k-tiles
                        ps_aux = psa_pool.tile([P, 512], F32)
                        nc.tensor.matmul(
                            ps_aux[0:1, S:2 * S], ones_col, a_t[:, 0, :],
                            start=True, stop=False)
                        nc.tensor.matmul(
                            ps_aux[0:1, S:2 * S], ones_col, a_t[:, 1, :],
                            start=False, stop=True)
                        rcp = aux_pool.tile([1, S], F32R, tag="rcp")
                        with nc.allow_low_precision(
                                reason="f32r softmax normalizer (fp22 ok)"):
                            nc.vector.reciprocal(rcp[:], ps_aux[0:1, S:2 * S])
                        # broadcast 1/denom across 128 partitions via PE
                        nc.tensor.matmul(ps_aux[:, 0:S], ones_row, rcp[:],
                                         start=True, stop=True)
                        rb = aux_pool.tile([P, S], F32, tag="rb")
                        nc.vector.tensor_copy(rb[:], ps_aux[:, 0:S])
                        # ctx^T[d', q] accumulated over k-tiles
                        ps_c = psc_pool.tile([P, S], F32)
                        nc.tensor.matmul(
                            ps_c[:], v_t[:, 0, hh * P:(hh + 1) * P],
                            a_t[:, 0, :], start=True, stop=False)
                        nc.tensor.matmul(
                            ps_c[:], v_t[:, 1, hh * P:(hh + 1) * P],
                            a_t[:, 1, :], start=False, stop=True)
                        nc.vector.tensor_mul(ctx_b[:, h, :], ps_c[:], rb[:])

                # out-projection for batch b: out[t, e] = ctx^T.T @ wo^T
                for tt in range(S // P):
                    for ec in range(D // 512):
                        ps_o = pso_pool.tile([P, 512], F32)
                        for h in range(H):
                            nc.tensor.matmul(
                                ps_o[:],
                                ctx_b[:, h, tt * P:(tt + 1) * P],
                                wo[:, h, ec * 512:(ec + 1) * 512],
                                start=(h == 0), stop=(h == H - 1),
                            )
                        o_t = o3_pool.tile([P, 512], F32)
                        nc.vector.tensor_copy(o_t[:], ps_o[:])
                        nc.sync.dma_start(
                            out=out[t0 + tt * P:t0 + (tt + 1) * P,
                                    ec * 512:(ec + 1) * 512],
                            in_=o_t[:],
                        )

    nc.compile()
    return nc


def get_nc(variant="full"):
    key = "nc:" + variant
    if key not in _CACHE:
        _CACHE[key] = _build(variant)
    return _CACHE[key]


def make_in_maps(x, w_qkv, w_o):
    x = np.ascontiguousarray(np.asarray(x, dtype=np.float32))
    w_qkv = np.asarray(w_qkv, dtype=np.float32)
    w_o = np.asarray(w_o, dtype=np.float32)
    # stripe layouts (see _build): element [*, p, dt, e] = w[e_glob, dt*128+p]
    wqk = np.ascontiguousarray(
        w_qkv[:2 * D].reshape(16, 256, DT, P).transpose(0, 3, 2, 1))
    wv = np.ascontiguousarray(
        w_qkv[2 * D:].reshape(4, 512, DT, P).transpose(0, 3, 2, 1))
    wo = np.ascontiguousarray(
        w_o.reshape(D, DT, P).transpose(2, 1, 0))
    # causal mask blocks: [tri(k<=q) | ones | zeros | tri]
    tri = np.triu(np.ones((P, P), dtype=np.float32))
    mask2 = np.concatenate(
        [tri, np.ones((P, P), np.float32), np.zeros((P, P), np.float32), tri],
        axis=1)
    in_maps = []
    for c in range(N_CORES):
        xs = x[c * NB:(c + 1) * NB].reshape(T, D)
        xt = np.ascontiguousarray(xs.reshape(T, DT, P).transpose(2, 1, 0))
        in_maps.append({"xt_t": xt, "wqk_t": wqk, "wv_t": wv, "wo_t": wo,
                        "mask2": mask2})
    return in_maps


def run(x, w_qkv, w_o, trace=False):
    nc = get_nc()
    in_maps = make_in_maps(x, w_qkv, w_o)
    res = run_bass_kernel_spmd(nc, in_maps, list(range(N_CORES)), trace=trace)
    outs = [res.results[i]["out"].reshape(NB, S, D) for i in range(N_CORES)]
    return np.concatenate(outs, axis=0), res


def kernel(**inputs):
    out, _ = run(inputs["x"], inputs["w_qkv"], inputs["w_o"])
    return out


def bench(x, w_qkv, w_o, iters=30, variant="full"):
    """Per-execution device time via chained donated executions.

    Chains n executions (outputs donated as the next call's buffers) under a
    single blocking wait; per-iteration time comes from the slope between two
    chain lengths, cancelling the ~80 ms axon RPC floor.
    Returns (per_iter_ns, floor_ns, outputs_full).
    """
    import time

    import jax
    from jax.sharding import Mesh, PartitionSpec, NamedSharding
    from jax.experimental.shard_map import shard_map

    import concourse.mybir as mb
    from concourse import bass2jax

    nc = get_nc(variant)
    bass2jax.install_neuronx_cc_hook()
    in_maps = make_in_maps(x, w_qkv, w_o)
    if variant == "p23":
        for m in in_maps:
            m["qkT_d"] = np.zeros((NB, ET, P, S), np.float32)
            m["v_d"] = np.zeros((T // P, 4, P, 512), np.float32)

    partition_name = (nc.partition_id_tensor.name
                      if nc.partition_id_tensor else None)
    in_names, out_names, out_avals, zero_outs = [], [], [], []
    for alloc in nc.m.functions[0].allocations:
        if not isinstance(alloc, mb.MemoryLocationSet):
            continue
        name = alloc.memorylocations[0].name
        if alloc.kind == "ExternalInput":
            if name != partition_name:
                in_names.append(name)
        elif alloc.kind == "ExternalOutput":
            out_names.append(name)
            shape = tuple(alloc.tensor_shape)
            dtype = mb.dt.np(alloc.dtype)
            out_avals.append(jax.core.ShapedArray(shape, dtype))
            zero_outs.append(np.zeros(shape, dtype))
    n_params = len(in_names)
    n_outs = len(out_names)
    all_names = in_names + out_names
    if partition_name is not None:
        all_names = all_names + [partition_name]
    donate = tuple(range(n_params, n_params + n_outs))

    def _body(*args):
        operands = list(args)
        if partition_name is not None:
            operands.append(bass2jax.partition_id_tensor())
        outs = bass2jax._bass_exec_p.bind(
            *operands,
            out_avals=tuple(out_avals),
            in_names=tuple(all_names),
            out_names=tuple(out_names),
            lowering_input_output_aliases=(),
            sim_require_finite=True,
            sim_require_nnan=True,
            nc=nc,
        )
        return tuple(outs)

    devices = jax.devices()[:N_CORES]
    mesh = Mesh(np.asarray(devices), ("core",))
    n_all = n_params + n_outs
    sharded = jax.jit(
        shard_map(_body, mesh=mesh,
                  in_specs=(PartitionSpec("core"),) * n_all,
                  out_specs=(PartitionSpec("core"),) * n_outs,
                  check_rep=False),
        donate_argnums=donate,
        keep_unused=True,
    )
    sharding = NamedSharding(mesh, PartitionSpec("core"))
    concat_in = [
        np.concatenate([in_maps[c][nm] for c in range(N_CORES)], axis=0)
        for nm in in_names
    ]
    concat_zeros = [np.zeros((N_CORES * z.shape[0], *z.shape[1:]), z.dtype)
                    for z in zero_outs]
    dev_in = [jax.device_put(a, sharding) for a in concat_in]

    def chain(n):
        dev_zeros = [jax.device_put(a, sharding) for a in concat_zeros]
        jax.block_until_ready(dev_zeros)
        t0 = time.perf_counter()
        outs = sharded(*dev_in, *dev_zeros)
        for _ in range(n - 1):
            outs = sharded(*dev_in, *outs)
        jax.block_until_ready(outs)
        return time.perf_counter() - t0, outs

    chain(1)  # warm-up / compile
    lo_n, hi_n = 2, 2 + iters
    lo = min(chain(lo_n)[0] for _ in range(5))
    hi_t, outs = chain(hi_n)
    for _ in range(4):
        t2, o2 = chain(hi_n)
        if t2 < hi_t:
            hi_t, outs = t2, o2
    per_iter_ns = (hi_t - lo) / (hi_n - lo_n) * 1e9
    print(f"bench: chain({lo_n}) {lo*1e3:.2f} ms, chain({hi_n}) "
          f"{hi_t*1e3:.2f} ms")
    out_np = np.asarray(outs[out_names.index("out")])
    per_core = out_np.reshape(N_CORES, T, D)
    full = np.concatenate([per_core[c].reshape(NB, S, D)
                           for c in range(N_CORES)], axis=0)
    return per_iter_ns, 0.0, full
